# revision 1
# baseline (speedup 1.0000x reference)
"""KAN layer (B=8192, IN_F=OUT_F=1024, GRID=5) on 8 Trainium2 cores.

Math: Y[b,o] = W0[o]*silu(x) + W1[o]*spline_o(clip(x,-1,1)) + b[o], x = X[b,o]
(idx_in = arange(O) % IN_F is the identity here since O == IN_F).

The degree-1 B-spline on the uniform 5-knot grid over [-1,1] is rewritten in
the relu basis: spline(xc) = c0 + m0*(xc+1) + sum_j (m_j - m_{j-1})*relu(xc - s_j)
with slopes m_g = 2*(c_{g+1}-c_g) and interior knots s_j in {-0.5, 0, 0.5}.
Folding W1 and b gives  Y^T[o,:] = W0*silu + B'*xc + G1*r1 + G2*r2 + G3*r3 + A'.

Layout: edges on SBUF partitions (X pre-transposed on host), batch on the free
dim, data-parallel over batch across the 8 cores.  The per-edge weighted sum of
the 5 feature maps runs on TensorE as 5 diagonal-stationary matmuls (silu in
fp32r, the four spline features in fp16) accumulating in PSUM; ScalarE
evacuates PSUM adding the per-edge bias A'.  Diagonal stationaries are built
on-device (identity * per-partition weight).  DMA: per-block input loads on the
Sync HWDGE queue, output stores on GpSimd SWDGE — ScalarE issues no DMAs.
"""
import sys

for _p in ("/root/.axon_site", "/root/.axon_site/_ro/trn_rl_repo", "/root/.axon_site/_ro/pypackages"):
    if _p not in sys.path:
        sys.path.append(_p)

import numpy as np

import concourse.bacc as bacc
import concourse.tile as tile
from concourse import mybir
from concourse.bass_utils import run_bass_kernel_spmd

B, IN_F, OUT_F, GRID = 8192, 1024, 1024, 5
N_CORES = 8
B_SHARD = B // N_CORES          # 1024 batch rows per core
EB = OUT_F // 128               # 8 edge blocks
NF = 5                          # features: silu, xc, r1, r2, r3
CHUNK = 512                     # one PSUM bank of fp32

# cpack layout (fp32 columns): [0:128] identity, [128:168] wT (5 weights x 8
# blocks, feature-major per block), [168:176] A'
WOFF, AOFF, CCOLS = 128, 168, 176

_nc_cache = None


def _build():
    f32 = mybir.dt.float32
    f32r = mybir.dt.float32r
    f16 = mybir.dt.float16
    AF = mybir.ActivationFunctionType
    OP = mybir.AluOpType
    nc = bacc.Bacc("TRN2", target_bir_lowering=False, debug=False)
    xt = nc.dram_tensor("xt", [OUT_F, B_SHARD], f32, kind="ExternalInput").ap()
    cpack = nc.dram_tensor("cpack", [128, CCOLS], f32, kind="ExternalInput").ap()
    yt = nc.dram_tensor("yt", [OUT_F, B_SHARD], f32, kind="ExternalOutput").ap()

    xt3 = xt.rearrange("(n p) d -> p n d", p=128)   # [128, EB, B_SHARD]
    yt3 = yt.rearrange("(n p) d -> p n d", p=128)

    with tile.TileContext(nc) as tc:
        with tc.tile_pool(name="const", bufs=1) as const_pool, \
             tc.tile_pool(name="xin", bufs=4) as xin_pool, \
             tc.tile_pool(name="feat", bufs=3) as feat_pool, \
             tc.tile_pool(name="feat0", bufs=1) as feat0_pool, \
             tc.tile_pool(name="yout", bufs=3) as yout_pool, \
             tc.tile_pool(name="ps", bufs=3, space="PSUM") as psum_pool, \
             tc.tile_pool(name="pswarm", bufs=1, space="PSUM") as warm_pool:
            cp = const_pool.tile([128, CCOLS], f32)
            nc.sync.dma_start(cp[:], cpack[:, :])
            ident32 = cp[:, 0:128]
            wv = cp[:, WOFF:WOFF + 40]               # [128, 40] fp32 weights
            ident16 = const_pool.tile([128, 128], f16)
            nc.vector.tensor_copy(ident16[:], ident32)

            # HAM warm-up: ~4.5us of dummy matmuls on uninitialized SBUF so
            # the PE clock-gate opens before the first real matmul arrives
            scratch = const_pool.tile([128, CHUNK], f16)
            nc.vector.memset(scratch[:], 0.0)
            ps_warm = warm_pool.tile([128, CHUNK], f32, tag="pswarm", name="pswarm")
            for _ in range(9):
                nc.tensor.matmul(ps_warm[:], scratch[:, 0:128], scratch[:],
                                 start=True, stop=True, skip_group_check=True)

            # per-block diagonal stationaries, built on device (emitted inside
            # the block loop so the pipeline starts immediately)
            dsilu = const_pool.tile([128, EB * 128], f32r)
            dspl = const_pool.tile([128, EB * 4 * 128], f16)

            def feature_ops(xv, n, tagsuf):
                """xv: [128, n, B_SHARD] input view -> 5 feature tiles."""
                pool = feat0_pool if tagsuf else feat_pool
                silu_t = pool.tile([128, n, B_SHARD], f32r, tag="silu" + tagsuf,
                                        name=f"silu{tagsuf}")
                nc.scalar.activation(silu_t[:], xv, AF.Silu)
                xc_t = pool.tile([128, n, B_SHARD], f16, tag="xc" + tagsuf,
                                      name=f"xc{tagsuf}")
                nc.vector.tensor_scalar(xc_t[:], xv, 1.0, -1.0, OP.min, OP.max)
                r1_t = pool.tile([128, n, B_SHARD], f16, tag="r1" + tagsuf,
                                      name=f"r1{tagsuf}")
                nc.vector.tensor_scalar(r1_t[:], xc_t[:], 0.5, 0.0, OP.add, OP.max)
                r2_t = pool.tile([128, n, B_SHARD], f16, tag="r2" + tagsuf,
                                      name=f"r2{tagsuf}")
                nc.vector.tensor_scalar_max(r2_t[:], xc_t[:], 0.0)
                r3_t = pool.tile([128, n, B_SHARD], f16, tag="r3" + tagsuf,
                                      name=f"r3{tagsuf}")
                nc.vector.tensor_scalar(r3_t[:], xc_t[:], -0.5, 0.0, OP.add, OP.max)
                return silu_t, xc_t, r1_t, r2_t, r3_t

            def block_matmuls(e, feats, hh, yo):
                """Build diags for block e, run the 10 matmuls, evacuate."""
                silu_t, xc_t, r1_t, r2_t, r3_t = feats
                ds = dsilu[:, e * 128:(e + 1) * 128]
                nc.vector.tensor_scalar_mul(ds, ident32, wv[:, e * NF:e * NF + 1])
                for j in range(4):
                    nc.vector.tensor_scalar_mul(
                        dspl[:, (e * 4 + j) * 128:(e * 4 + j + 1) * 128],
                        ident16[:], wv[:, e * NF + 1 + j:e * NF + 2 + j])
                ps = psum_pool.tile([128, B_SHARD], f32, tag="ps", name=f"ps_{e}")

                # xc is ready before silu (clip is cheaper than the ACT pass),
                # so start each block's accumulation with the spline features
                # and finish with silu
                def block_chunk(ts):
                    for j, ft in enumerate((xc_t, r1_t, r2_t, r3_t)):
                        for t in ts:
                            nc.tensor.matmul(ps[:, t * CHUNK:(t + 1) * CHUNK],
                                             dspl[:, (e * 4 + j) * 128:(e * 4 + j + 1) * 128],
                                             ft[:, hh, t * CHUNK:(t + 1) * CHUNK],
                                             start=(j == 0), stop=False,
                                             skip_group_check=True)
                    for t in ts:
                        nc.tensor.matmul(ps[:, t * CHUNK:(t + 1) * CHUNK], ds,
                                         silu_t[:, hh, t * CHUNK:(t + 1) * CHUNK],
                                         start=False, stop=True, skip_group_check=True)

                if e < EB - 1:
                    block_chunk((0, 1))
                    nc.scalar.activation(yo[:, e % 2, :], ps[:], AF.Identity,
                                         bias=cp[:, AOFF + e:AOFF + e + 1], scale=1.0)
                else:
                    # last block: per-chunk pipeline on VectorE for a short tail
                    for t in range(2):
                        block_chunk((t,))
                        nc.vector.tensor_scalar_add(
                            yo[:, e % 2, t * CHUNK:(t + 1) * CHUNK],
                            ps[:, t * CHUNK:(t + 1) * CHUNK],
                            cp[:, AOFF + e:AOFF + e + 1])

            for ep in range(EB // 2):
                if ep == 0:
                    # first pair: per-block DMAs and per-block features so
                    # compute starts as soon as 512 KB has landed
                    yo = yout_pool.tile([128, 2, B_SHARD], f32, tag="yo", name="yo_p0")
                    for h in range(2):
                        x_t = xin_pool.tile([128, 1, B_SHARD], f32, tag=f"x0{h}",
                                            name=f"x0{h}")
                        nc.sync.dma_start(x_t[:], xt3[:, h:h + 1, :])
                        feats = feature_ops(x_t[:], 1, f"0{h}")
                        block_matmuls(h, feats, 0, yo)
                else:
                    x_t = xin_pool.tile([128, 2, B_SHARD], f32, tag="x",
                                        name=f"x_p{ep}")
                    nc.sync.dma_start(x_t[:], xt3[:, 2 * ep:2 * ep + 2, :])
                    feats = feature_ops(x_t[:], 2, "")
                    yo = yout_pool.tile([128, 2, B_SHARD], f32, tag="yo",
                                        name=f"yo_p{ep}")
                    for h in range(2):
                        block_matmuls(2 * ep + h, feats, h, yo)
                if ep == EB // 2 - 1:
                    # split the last stores across two queues for a short tail
                    nc.gpsimd.dma_start(yt3[:, 2 * ep:2 * ep + 1, :], yo[:, 0:1, :])
                    nc.gpsimd.dma_start(yt3[:, 2 * ep + 1:2 * ep + 2, 0:CHUNK],
                                        yo[:, 1:2, 0:CHUNK])
                    nc.sync.dma_start(yt3[:, 2 * ep + 1:2 * ep + 2, CHUNK:B_SHARD],
                                      yo[:, 1:2, CHUNK:B_SHARD])
                elif ep % 2 == 0:
                    nc.gpsimd.dma_start(yt3[:, 2 * ep:2 * ep + 2, :], yo[:])
                else:
                    nc.sync.dma_start(yt3[:, 2 * ep:2 * ep + 2, :], yo[:])
    nc.compile()
    return nc


def _host_prep(X, coeffs, W, b):
    c = coeffs.astype(np.float64)
    W = W.astype(np.float64)
    b = b.astype(np.float64)
    m = 2.0 * (c[:, 1:] - c[:, :-1])            # [O, 4] slopes per unit xc
    w1 = W[:, 1]
    aprime = w1 * (c[:, 0] + m[:, 0]) + b        # const term (incl. m0*(xc+1) fold)
    bprime = w1 * m[:, 0]
    g = w1[:, None] * (m[:, 1:] - m[:, :-1])     # [O, 3] relu weights at s=-0.5,0,0.5
    wvec = np.stack([W[:, 0], bprime, g[:, 0], g[:, 1], g[:, 2]], axis=1)  # [O, 5]

    cpack = np.zeros((128, CCOLS), dtype=np.float32)
    cpack[:, 0:128] = np.eye(128, dtype=np.float32)
    for e in range(EB):
        for f in range(NF):
            cpack[:, WOFF + e * NF + f] = wvec[e * 128:(e + 1) * 128, f].astype(np.float32)
        cpack[:, AOFF + e] = aprime[e * 128:(e + 1) * 128].astype(np.float32)
    return cpack


def kernel(X, coeffs, W, b):
    global _nc_cache
    if _nc_cache is None:
        _nc_cache = _build()
    nc = _nc_cache

    cpack = _host_prep(X, coeffs, W, b)
    in_maps = []
    for c in range(N_CORES):
        xt_shard = np.ascontiguousarray(X[c * B_SHARD:(c + 1) * B_SHARD, :].T)
        in_maps.append({"xt": xt_shard, "cpack": cpack})

    res = run_bass_kernel_spmd(nc, in_maps, core_ids=list(range(N_CORES)))
    Y = np.empty((B, OUT_F), dtype=np.float32)
    for c in range(N_CORES):
        Y[c * B_SHARD:(c + 1) * B_SHARD, :] = res.results[c]["yt"].T.astype(np.float32)
    return Y



# revision 2
# speedup vs baseline: 1.0279x; 1.0279x over previous
"""KAN layer (B=8192, IN_F=OUT_F=1024, GRID=5) on 8 Trainium2 cores.

Math: Y[b,o] = W0[o]*silu(x) + spline_o(clip(x,-1,1)) + b[o], x = X[b,o].
The degree-1 B-spline is evaluated in the *segment* basis
    spline(clip(x)) = A''[o] + sum_j gamma_j[o] * v_j(x),
    v_j(x) = clip(x, s_{j-1}, s_j),  knots s = (-1,-0.5,0,0.5,1),
    gamma_j = w1 * m_j (segment slopes),
so each map is a 2-op tensor_scalar clip straight from x.

Sharding: edges across the 8 cores (128 edges/core, full batch 8192 on the
free dim).  Per core TensorE does a per-edge diagonal combine of 5 feature
maps into PSUM: v2,v3 ride ONE fp8e4 DoubleRow matmul (2 maps/pass), v1,v4
and silu are fp16 matmuls.  ScalarE: silu + most of the PSUM evacuation
(Identity+bias); VectorE: the 4 clips + the evac remainder.  I/O is fp16
(host converts); fp8 weight-quantization error is minimax-compensated into
the per-edge bias on host.
"""
import sys

for _p in ("/root/.axon_site", "/root/.axon_site/_ro/trn_rl_repo", "/root/.axon_site/_ro/pypackages"):
    if _p not in sys.path:
        sys.path.append(_p)

import numpy as np
import ml_dtypes

import concourse.bacc as bacc
import concourse.tile as tile
from concourse import mybir
from concourse.bass_utils import run_bass_kernel_spmd

B, IN_F, OUT_F, GRID = 8192, 1024, 1024, 5
N_CORES = 8
PER = OUT_F // N_CORES          # 128 edges per core
NB = B                          # 8192 batch columns per core
SBLK = 2048                     # superblock columns
NSB = NB // SBLK                # 4 superblocks
CHUNK = 512                     # one PSUM bank of fp32
SPLIT = 1920                    # evac columns done on ScalarE (rest on VectorE)

_nc_cache = None


def _build():
    f32 = mybir.dt.float32
    f16 = mybir.dt.float16
    f8 = mybir.dt.float8e4
    AF = mybir.ActivationFunctionType
    OP = mybir.AluOpType
    DRm = mybir.MatmulPerfMode.DoubleRow

    nc = bacc.Bacc("TRN2", target_bir_lowering=False, debug=False)
    xt = nc.dram_tensor("xt", [PER, NB], f16, kind="ExternalInput").ap()
    cpack = nc.dram_tensor("cpack", [PER, 8], f32, kind="ExternalInput").ap()
    ident = nc.dram_tensor("ident", [PER, 128], f16, kind="ExternalInput").ap()
    yt = nc.dram_tensor("yt", [PER, NB], f16, kind="ExternalOutput").ap()

    with tile.TileContext(nc) as tc:
        with tc.tile_pool(name="const", bufs=1) as cpool, \
             tc.tile_pool(name="xin", bufs=1) as xpool, \
             tc.tile_pool(name="sil", bufs=2) as spool, \
             tc.tile_pool(name="v14", bufs=2) as vpool, \
             tc.tile_pool(name="v23", bufs=2) as wpool, \
             tc.tile_pool(name="yout", bufs=2) as ypool, \
             tc.tile_pool(name="ps", bufs=2, space="PSUM") as pspool:
            # consts + scratch first so every engine has early work
            cp = cpool.tile([128, 8], f32)
            nc.gpsimd.dma_start(cp[:], cpack[:, :])
            id16 = cpool.tile([128, 128], f16)
            nc.gpsimd.dma_start(id16[:], ident[:, :])

            scr = cpool.tile([128, CHUNK], f16)
            nc.vector.memset(scr[:], 0.25)
            # silu ACT-table load overlaps the first input DMA
            dum = cpool.tile([128, 1], f16)
            nc.scalar.activation(dum[:], scr[:, 0:1], AF.Silu)

            # input loads (SWDGE/gpsimd ring; issued before VectorE gets busy)
            x0 = xpool.tile([128, SBLK], f16, tag="x0", name="x0")
            nc.gpsimd.dma_start(x0[:], xt[:, 0:SBLK])
            x1 = xpool.tile([128, SBLK], f16, tag="x1", name="x1")
            nc.gpsimd.dma_start(x1[:], xt[:, SBLK:2 * SBLK])
            x23 = xpool.tile([128, 2 * SBLK], f16, tag="x23", name="x23")
            nc.gpsimd.dma_start(x23[:], xt[:, 2 * SBLK:4 * SBLK])

            # PE warm-up: ~4.3us of matmuls on scratch so HAM reaches 8/8
            pswarm = pspool.tile([128, SBLK], f32, tag="ps", name="pswarm")
            for r in range(10):
                nc.tensor.matmul(pswarm[:, 0:CHUNK], scr[:, 0:128], scr[:],
                                 start=True, stop=True, skip_group_check=True)

            # per-edge diagonal stationaries (on-device from ident * weight)
            dsil = cpool.tile([128, 128], f16)
            nc.vector.tensor_scalar_mul(dsil[:], id16[:], cp[:, 0:1])
            dv1 = cpool.tile([128, 128], f16)
            nc.vector.tensor_scalar_mul(dv1[:], id16[:], cp[:, 1:2])
            dp23 = cpool.tile([128, 2, 128], f8)
            nc.vector.tensor_scalar_mul(dp23[:, 0, :], id16[:], cp[:, 2:3])
            nc.vector.tensor_scalar_mul(dp23[:, 1, :], id16[:], cp[:, 3:4])
            dv4 = cpool.tile([128, 128], f16)
            nc.vector.tensor_scalar_mul(dv4[:], id16[:], cp[:, 4:5])

            for j in range(NSB):
                if j == 0:
                    xv = x0[:]
                elif j == 1:
                    xv = x1[:]
                else:
                    xv = x23[:, (j - 2) * SBLK:(j - 1) * SBLK]

                sil = spool.tile([128, SBLK], f16, tag="sil", name=f"sil{j}")
                nc.scalar.activation(sil[:], xv, AF.Silu)
                v23 = wpool.tile([128, 2, SBLK], f8, tag="v23", name=f"v23_{j}")
                nc.vector.tensor_scalar(v23[:, 0, :], xv, 0.0, -0.5, OP.min, OP.max)
                nc.vector.tensor_scalar(v23[:, 1, :], xv, 0.5, 0.0, OP.min, OP.max)
                v1 = vpool.tile([128, SBLK], f16, tag="v1", name=f"v1_{j}")
                nc.vector.tensor_scalar(v1[:], xv, -0.5, -1.0, OP.min, OP.max)
                v4 = vpool.tile([128, SBLK], f16, tag="v4", name=f"v4_{j}")
                nc.vector.tensor_scalar(v4[:], xv, 1.0, 0.5, OP.min, OP.max)

                ps = pspool.tile([128, SBLK], f32, tag="ps", name=f"ps{j}")
                for c in range(4):
                    nc.tensor.matmul(ps[:, c * CHUNK:(c + 1) * CHUNK],
                                     dp23[:, 0:2, :],
                                     v23[:, 0:2, c * CHUNK:(c + 1) * CHUNK],
                                     start=True, stop=False, perf_mode=DRm,
                                     skip_group_check=True)
                for c in range(4):
                    nc.tensor.matmul(ps[:, c * CHUNK:(c + 1) * CHUNK], dv1[:],
                                     v1[:, c * CHUNK:(c + 1) * CHUNK],
                                     start=False, stop=False, skip_group_check=True)
                for c in range(4):
                    nc.tensor.matmul(ps[:, c * CHUNK:(c + 1) * CHUNK], dv4[:],
                                     v4[:, c * CHUNK:(c + 1) * CHUNK],
                                     start=False, stop=False, skip_group_check=True)
                for c in range(4):
                    nc.tensor.matmul(ps[:, c * CHUNK:(c + 1) * CHUNK], dsil[:],
                                     sil[:, c * CHUNK:(c + 1) * CHUNK],
                                     start=False, stop=True, skip_group_check=True)

                y = ypool.tile([128, SBLK], f16, tag="y", name=f"y{j}")
                nc.scalar.activation(y[:, 0:SPLIT], ps[:, 0:SPLIT], AF.Identity,
                                     bias=cp[:, 5:6], scale=1.0)
                nc.vector.tensor_scalar(y[:, SPLIT:SBLK], ps[:, SPLIT:SBLK],
                                        cp[:, 5:6], None, OP.add)

                if j < NSB - 1:
                    nc.sync.dma_start(yt[:, j * SBLK:(j + 1) * SBLK], y[:])
                else:
                    nc.sync.dma_start(yt[:, j * SBLK:j * SBLK + 1536], y[:, 0:1536])
                    nc.sync.dma_start(yt[:, j * SBLK + 1536:(j + 1) * SBLK],
                                      y[:, 1536:SBLK])
    nc.compile()
    return nc


def _host_prep(X, coeffs, W, b):
    """Per-core cpack [128, 8] fp32: W0, g1, g2, g3, g4, A'' (compensated)."""
    c = coeffs.astype(np.float64)
    W64 = W.astype(np.float64)
    b64 = b.astype(np.float64)
    m = 2.0 * (c[:, 1:] - c[:, :-1])          # [O, 4] segment slopes
    w1 = W64[:, 1]
    gam = w1[:, None] * m                      # [O, 4]
    s = np.array([-1.0, -0.5, 0.0, 0.5])
    A = b64 + w1 * c[:, 0] - (gam * s[None, :]).sum(1)
    # minimax compensation of fp8e4 quantization of g2, g3 (device uses RNE)
    d2 = gam[:, 1].astype(ml_dtypes.float8_e4m3).astype(np.float64) - gam[:, 1]
    d3 = gam[:, 2].astype(ml_dtypes.float8_e4m3).astype(np.float64) - gam[:, 2]
    cand = np.stack([-0.5 * d2, np.zeros_like(d2), 0.5 * d3], 1)
    A = A - (cand.max(1) + cand.min(1)) / 2

    cpack = np.zeros((OUT_F, 8), dtype=np.float32)
    cpack[:, 0] = W64[:, 0]
    cpack[:, 1:5] = gam
    cpack[:, 5] = A
    return cpack


def kernel(X, coeffs, W, b):
    global _nc_cache
    if _nc_cache is None:
        _nc_cache = _build()
    nc = _nc_cache

    cpack = _host_prep(X, coeffs, W, b)
    ident = np.eye(128, dtype=np.float16)
    X16 = X.astype(np.float16)
    in_maps = []
    for cidx in range(N_CORES):
        sl = slice(cidx * PER, (cidx + 1) * PER)
        in_maps.append({
            "xt": np.ascontiguousarray(X16[:, sl].T),
            "cpack": np.ascontiguousarray(cpack[sl]),
            "ident": ident,
        })

    res = run_bass_kernel_spmd(nc, in_maps, core_ids=list(range(N_CORES)))
    Y = np.empty((B, OUT_F), dtype=np.float32)
    for cidx in range(N_CORES):
        sl = slice(cidx * PER, (cidx + 1) * PER)
        Y[:, sl] = res.results[cidx]["yt"].T.astype(np.float32)
    return Y
